# revision 23
# baseline (speedup 1.0000x reference)
"""Expert-parallel MoE GroupedMLP kernel for 8 Trainium2 NeuronCores.

Problem: T=4096 tokens, H=2048 hidden, E=8 experts, I=4096 intermediate,
top_k=2, fp32 reference.

Strategy (balanced multi-slot expert-parallel, sharded inside kernel()):
  - Host: softmax + top-k routing.  The token->core assignment is load-
    balanced: every core runs the SAME program with k (2 or 3) fixed-size
    single-expert token slots; which expert each slot serves is carried
    entirely by input data (per-slot weight tensors + gathered tokens), so
    one SPMD program covers an arbitrary expert->core packing.  Slot sizes
    are solved from the routing counts by a small exact DP; slots are kept
    >= ~300 tokens so each slot's full-weight stream (~50 MB) stays hidden
    under its compute.
  - Device: tokens are the matmul moving dim in BOTH matmuls (w1 and w2
    are the stationary operands), so slot sizes need no 128 alignment.
    bf16 matmuls, fp32 PSUM, SiLU on the scalar engine, per-token
    combine-weight scaling on the vector engine.  Weights are host-
    pretiled into the exact SBUF tile layout for contiguous HBM reads;
    weight streams ride the sync HWDGE queue, tokens/combine/outputs the
    scalar HWDGE queue.
  - Host: scatter-add the per-slot [H, len] outputs into the [T, H] result.
"""

import time

import numpy as np
import ml_dtypes

from concourse import bass, bacc, tile, mybir
from concourse.bass_utils import run_bass_kernel_spmd

# Problem dims (hardcoded per contract)
T, H, E, I = 4096, 2048, 8, 4096
P = 128          # partitions
KH = H // P      # 16 contraction tiles for MM1
NJ = I // P      # 32 intermediate j-tiles (acts)
SL = I // 256    # 16 w1 slabs of 256 cols per gate/up half
NH = H // P      # 16 output h-tiles
NCORES = 8
BMIN = 304       # min slot size: hide the ~50MB/slot weight stream

_BF16 = mybir.dt.bfloat16
_F32 = mybir.dt.float32


def _split512(n):
    """Split n tokens into matmul chunks of <=512 (PSUM bank limit)."""
    out = []
    while n > 512:
        out.append(512)
        n -= 512
    if n:
        out.append(n)
    return tuple(out)


def _pad4(n):
    # pad to even token counts: keeps every DMA row stride 4-byte aligned
    # (bf16 rows = 2L*2 B, f32 rows = 4L B) while wasting at most 1 token
    return -(-n // 2) * 2


def solve_slots2(counts, ncores=NCORES, bmin=BMIN, cap=None):
    """Slot sizes (A, B), A >= B, minimizing A+B such that the expert
    token counts are covered by <=ncores slots of each size (an expert may
    use several slots; slots may be left empty).  Bitmask DP feasibility."""
    counts = [int(c) for c in counts]
    total = sum(counts)
    maxn = max(counts)
    s_lo = max(-(-total // ncores), 16)
    sup = ncores + 1

    def feasible(A, B):
        state = np.zeros((sup, sup), dtype=bool)
        state[ncores, ncores] = True
        for n in counts:
            new = np.zeros_like(state)
            for alpha in range(ncores + 1):
                rem = n - alpha * A
                beta = 0 if rem <= 0 else -(-rem // B)
                if beta > ncores:
                    continue
                if beta:
                    new[:sup - alpha or None, :sup - beta] |= \
                        state[alpha:, beta:]
                else:
                    new[:sup - alpha or None, :] |= state[alpha:, :]
                if rem <= 0:
                    break
            state = new
            if not state.any():
                return False
        return True

    def recover(A, B):
        from functools import lru_cache

        @lru_cache(maxsize=None)
        def dfs(e, sa, sb):
            if e == len(counts):
                return ()
            n = counts[e]
            for alpha in range(sa + 1):
                rem = n - alpha * A
                beta = 0 if rem <= 0 else -(-rem // B)
                if beta > sb:
                    continue
                rest = dfs(e + 1, sa - alpha, sb - beta)
                if rest is not None:
                    return ((alpha, beta),) + rest
            return None

        return dfs(0, ncores, ncores)

    for S in range(s_lo, 4 * maxn + 64):
        a_hi = max(-(-S // 2), S - bmin)         # keep B >= bmin if possible
        for A in range(-(-S // 2), a_hi + 1):
            B = S - A
            if B < 8 or (cap and A > cap):
                continue
            if feasible(A, B):
                pat = recover(A, B)
                return (A, B), [tuple(p) for p in pat]
    return None


def solve_slots3(counts, ncores=NCORES, bmin=BMIN, s_max=None,
                 budget_s=45.0):
    """Three slot sizes (a >= b >= c >= bmin) minimizing a+b+c, same cover
    rules.  Returns None if infeasible within bounds/budget."""
    counts = [int(c) for c in counts]
    total = sum(counts)
    maxn = max(counts)
    s_lo = max(-(-total // ncores), 3 * bmin)
    if s_max is None:
        s_max = 4 * maxn
    sup = ncores + 1
    t0 = time.time()

    def feasible(a, b, c):
        state = np.zeros((sup, sup, sup), dtype=bool)
        state[ncores, ncores, ncores] = True
        for n in counts:
            new = np.zeros_like(state)
            for al in range(ncores + 1):
                r1 = n - al * a
                for be in range(ncores + 1):
                    r2 = r1 - be * b
                    ga = 0 if r2 <= 0 else -(-r2 // c)
                    if ga > ncores:
                        continue
                    if ga:
                        new[:sup - al or None, :sup - be or None,
                            :sup - ga] |= state[al:, be:, ga:]
                    elif be:
                        new[:sup - al or None, :sup - be, :] |= \
                            state[al:, be:, :]
                    else:
                        new[:sup - al or None, :, :] |= state[al:, :, :]
                    if r1 <= 0:
                        break
                state_any = True
            state = new
            if not state.any():
                return False
        return True

    def recover(a, b, c):
        from functools import lru_cache

        @lru_cache(maxsize=None)
        def dfs(e, sa, sb, sc):
            if e == len(counts):
                return ()
            n = counts[e]
            for al in range(sa + 1):
                r1 = n - al * a
                for be in range(sb + 1):
                    r2 = r1 - be * b
                    ga = 0 if r2 <= 0 else -(-r2 // c)
                    if ga > sc:
                        continue
                    rest = dfs(e + 1, sa - al, sb - be, sc - ga)
                    if rest is not None:
                        return ((al, be, ga),) + rest
                    if r1 <= 0:
                        break
            return None

        return dfs(0, ncores, ncores, ncores)

    for S in range(s_lo, s_max):
        for a in range(-(-S // 3), S - 2 * bmin + 1):
            for b in range(max(bmin, -(-(S - a) // 2)),
                           min(a, S - a - bmin) + 1):
                c = S - a - b
                if c < bmin or c > b:
                    continue
                if time.time() - t0 > budget_s:
                    return None
                if feasible(a, b, c):
                    pat = recover(a, b, c)
                    if pat is not None:
                        return (a, b, c), [tuple(p) for p in pat]
    return None


def solve_slots(counts):
    """Best slot plan: try 3 slots, fall back to 2.  Returns (sizes, pat)
    with sizes already padded to a multiple of 4."""
    counts = [int(c) for c in counts]
    r2 = solve_slots2(counts)
    s2 = sum(r2[0]) if r2 else 1 << 30
    r3 = solve_slots3(counts, s_max=min(s2, 4 * max(counts)))
    best = None
    if r3 is not None and sum(_pad4(s) for s in r3[0]) < \
            (sum(_pad4(s) for s in r2[0]) if r2 else 1 << 30):
        best = r3
    elif r2 is not None:
        best = r2
    else:
        # trivial always-feasible fallback: one big slot per expert
        A = max(counts)
        best = ((A, A), [(1, 1) for _ in counts])
    sizes, pat = best
    padded = tuple(_pad4(s) for s in sizes)
    # PSUM budget (shared [128,512] phase-A psum tags, bufs>=1): per-slot
    # chunk count must stay <= 3 so 2*chunks + 2 phase-B banks fit in 8
    if max(len(_split512(p)) for p in padded) > 3:
        # extremely skewed routing: cap slot sizes at 512 and re-solve
        sizes, pat = solve_slots2(counts, cap=512)
        padded = tuple(_pad4(s) for s in sizes)
    return sizes, padded, pat


def build_kernel(slot_chunks):
    """One SPMD program: len(slot_chunks) single-expert token slots of
    fixed sizes.  Slot weights / tokens / combine-weights are inputs;
    output per slot is y[H, len] (tokens free dim), combine-scaled."""
    nslots = len(slot_chunks)
    lens = [sum(ch) for ch in slot_chunks]
    Amax = max(lens)
    nc = bacc.Bacc("TRN2", target_bir_lowering=False, debug=False,
                   num_devices=NCORES)
    dts = []
    for i, L in enumerate(lens):
        dts.append((
            nc.dram_tensor(f"x{i}", [KH // 2, P, 2 * L], _BF16,
                           kind="ExternalInput").ap(),
            nc.dram_tensor(f"w1_{i}", [2 * SL, P, KH * 256], _BF16,
                           kind="ExternalInput").ap(),
            nc.dram_tensor(f"w2_{i}", [NH, P, NJ * P], _BF16,
                           kind="ExternalInput").ap(),
            nc.dram_tensor(f"c{i}", [P, L], _F32,
                           kind="ExternalInput").ap(),
            nc.dram_tensor(f"y{i}", [H, L], _F32,
                           kind="ExternalOutput").ap(),
        ))

    AF = mybir.ActivationFunctionType
    # phase-A psum ping-pong (removes group-boundary stalls) when the
    # 8-bank budget allows: 2 tags x bufs x 1 bank + 2 phase-B banks
    max_chunks = max(len(ch) for ch in slot_chunks)
    psa_bufs = 2 if 2 * max_chunks * 2 + 2 <= 8 else 1

    with tile.TileContext(nc) as tc:
        with (
            tc.tile_pool(name="xp", bufs=1) as xp,
            tc.tile_pool(name="w1p", bufs=3) as w1p,
            tc.tile_pool(name="w2p", bufs=3) as w2p,
            tc.tile_pool(name="actp", bufs=1) as actp,
            tc.tile_pool(name="cp", bufs=1) as cp,
            tc.tile_pool(name="sp", bufs=2) as sp,
            tc.tile_pool(name="op", bufs=3) as op,
            tc.tile_pool(name="psA", bufs=psa_bufs, space="PSUM") as psA,
            tc.tile_pool(name="psB", bufs=2, space="PSUM") as psB,
        ):
            for si, chunks in enumerate(slot_chunks):
                x_d, w1_d, w2_d, c_d, y_d = dts[si]
                C = lens[si]
                offs = [sum(chunks[:i]) for i in range(len(chunks))]

                def load_slab(jp, halves=1):
                    # tile cols: lj*2048 + k*128 + nl (lj-major pack), so
                    # a half-split delivers the lj=0 matmul operands first
                    g = w1p.tile([P, KH * 256], _BF16, tag="w1g",
                                 name=f"w1g_{si}_{jp}")
                    u = w1p.tile([P, KH * 256], _BF16, tag="w1u",
                                 name=f"w1u_{si}_{jp}")
                    hw = KH * 256 // halves
                    for hh in range(halves):
                        hs = slice(hh * hw, (hh + 1) * hw)
                        nc.sync.dma_start(out=g[:, hs], in_=w1_d[2 * jp, :, hs])
                    for hh in range(halves):
                        hs = slice(hh * hw, (hh + 1) * hw)
                        nc.sync.dma_start(out=u[:, hs],
                                          in_=w1_d[2 * jp + 1, :, hs])
                    return g, u

                # first slab ahead of the token stream, lj-half pieces so
                # the first matmul group needs only the leading 1 MB
                gu0 = load_slab(0, halves=2 if si == 0 else 1)

                # this slot's tokens: one [128, 2C] tile per k-pair, on the
                # scalar HWDGE queue (parallel with the sync weight queue)
                xtiles = []            # (tile, column base) per k-tile
                for kk in range(KH // 2):
                    xk = xp.tile([P, 2 * C], _BF16, tag=f"x{si}_{kk}")
                    nc.scalar.dma_start(out=xk[:], in_=x_d[kk])
                    xtiles.append((xk, 0))
                    xtiles.append((xk, C))

                # ---- phase A: h1 = x @ w1.T ; act = silu(gate)*up ----
                acts = []
                for jp in range(SL):
                    g, u = gu0 if jp == 0 else load_slab(jp)
                    for lj in range(2):
                        j = jp * 2 + lj
                        pgs = [psA.tile([P, 512], _F32, tag=f"pg{c}",
                                        name=f"pg{c}_{si}_{j}")
                               for c, cl in enumerate(chunks)]
                        pus = [psA.tile([P, 512], _F32, tag=f"pu{c}",
                                        name=f"pu{c}_{si}_{j}")
                               for c, cl in enumerate(chunks)]
                        for k in range(KH):
                            ws = slice(lj * KH * P + k * P,
                                       lj * KH * P + k * P + P)
                            xt, xb = xtiles[k]
                            for c, cl in enumerate(chunks):
                                o = xb + offs[c]
                                nc.tensor.matmul(
                                    pgs[c][:, :cl], g[:, ws],
                                    xt[:, o:o + cl],
                                    start=(k == 0), stop=(k == KH - 1))
                        for k in range(KH):
                            ws = slice(lj * KH * P + k * P,
                                       lj * KH * P + k * P + P)
                            xt, xb = xtiles[k]
                            for c, cl in enumerate(chunks):
                                o = xb + offs[c]
                                nc.tensor.matmul(
                                    pus[c][:, :cl], u[:, ws],
                                    xt[:, o:o + cl],
                                    start=(k == 0), stop=(k == KH - 1))
                        at = actp.tile([P, Amax], _BF16, tag=f"act{j}",
                                       name=f"act{j}_{si}")
                        for c, cl in enumerate(chunks):
                            st = sp.tile([P, cl], _F32, tag="silu")
                            nc.scalar.activation(st[:], pgs[c][:, :cl],
                                                 AF.Sigmoid)
                            nc.vector.tensor_mul(st[:], st[:], pgs[c][:, :cl])
                            nc.vector.tensor_mul(
                                at[:, offs[c]:offs[c] + cl], st[:],
                                pus[c][:, :cl])
                        acts.append(at)

                # ---- phase B: y = combine * (act @ w2.T) ----
                ct = cp.tile([P, C], _F32, tag=f"c{si}")
                nc.scalar.dma_start(out=ct[:], in_=c_d[:])
                for h in range(NH):
                    wt = w2p.tile([P, NJ * P], _BF16, tag="w2",
                                  name=f"w2_{si}_{h}")
                    nc.sync.dma_start(out=wt[:], in_=w2_d[h])
                    for c, cl in enumerate(chunks):
                        po = psB.tile([P, cl], _F32, tag="po")
                        for j in range(NJ):
                            nc.tensor.matmul(
                                po[:], wt[:, j * P:(j + 1) * P],
                                acts[j][:, offs[c]:offs[c] + cl],
                                start=(j == 0), stop=(j == NJ - 1))
                        ot = op.tile([P, cl], _F32, tag="out")
                        nc.vector.tensor_mul(ot[:], po[:],
                                             ct[:, offs[c]:offs[c] + cl])
                        nc.scalar.dma_start(
                            out=y_d[h * P:(h + 1) * P, offs[c]:offs[c] + cl],
                            in_=ot[:])
    nc.compile()
    return nc


_NC_CACHE = {}
_WPACK_CACHE = {}
LAST_RESULTS = []   # BassKernelResults of each wave of the last kernel() call


def _get_nc(slot_chunks):
    if slot_chunks not in _NC_CACHE:
        _NC_CACHE[slot_chunks] = build_kernel(slot_chunks)
    return _NC_CACHE[slot_chunks]


def _pack_weights(w1, w2):
    """Pretile weights into the device tile layout (bf16, contiguous DMA).
    w1 [E, 2I, H] -> [E, 32, 128, 4096]: [e, 2*jp+s, p, lj*2048+k*128+nl]
      = w1[e, s*I + jp*256 + lj*128 + nl, k*128 + p]   (lj-major cols)
    w2 [E, H, I]  -> [E, 16, 128, 4096]: [e, h, p, j*128+hc] =
      w2[e, h*128+hc, j*128+p]
    """
    fp = (w1.shape, w2.shape, w1.ctypes.data, w2.ctypes.data,
          float(w1.flat[0]), float(w2.flat[0]), float(w1.flat[-1]))
    if _WPACK_CACHE.get("fp") == fp:
        return _WPACK_CACHE["w1"], _WPACK_CACHE["w2"]
    # [E, s, jp, lj, nl, k, p] -> [E, jp, s, p, lj, k, nl]
    w1p = np.ascontiguousarray(
        w1.reshape(E, 2, SL, 2, P, KH, P).transpose(0, 2, 1, 6, 3, 5, 4)
    ).astype(ml_dtypes.bfloat16).reshape(E, 2 * SL, P, KH * 256)
    w2p = np.ascontiguousarray(
        w2.reshape(E, NH, P, NJ, P).transpose(0, 1, 4, 3, 2)
    ).astype(ml_dtypes.bfloat16).reshape(E, NH, P, NJ * P)
    _WPACK_CACHE.update(fp=fp, w1=w1p, w2=w2p)
    return w1p, w2p


def _route(router_logits, top_k):
    """Host routing: stable softmax + top-k (ties broken by lower index,
    matching jax.lax.top_k)."""
    logits = np.asarray(router_logits, dtype=np.float32)
    m = logits.max(axis=-1, keepdims=True)
    p = np.exp(logits - m)
    p /= p.sum(axis=-1, keepdims=True)
    ids = np.argsort(-p, axis=-1, kind="stable")[:, :top_k]   # [T, k]
    gates = np.take_along_axis(p, ids, axis=-1)               # [T, k]
    return ids, gates


def kernel(hidden_states, router_logits, w1, w2, top_k):
    top_k = int(top_k)
    x = np.asarray(hidden_states, dtype=np.float32)
    w1 = np.asarray(w1, dtype=np.float32)
    w2 = np.asarray(w2, dtype=np.float32)
    n_tok, hidden = x.shape
    n_exp = w1.shape[0]
    assert (n_tok, hidden, n_exp) == (T, H, E), "compiled for fixed shapes"

    ids, gates = _route(router_logits, top_k)

    # per-expert token lists (sorted by expert, stable in token order)
    expert_of = ids.ravel()
    token_of = np.repeat(np.arange(n_tok, dtype=np.int64), top_k)
    gate_of = gates.ravel()
    order = np.argsort(expert_of, kind="stable")
    token_sorted = token_of[order]
    gate_sorted = gate_of[order]
    counts = np.bincount(expert_of, minlength=n_exp)
    starts = np.concatenate([[0], np.cumsum(counts)])

    live = [int(c) for c in counts if c > 0]
    live_idx = [e for e in range(n_exp) if counts[e] > 0]
    sizes, padded, pat_live = solve_slots(live)
    nslots = len(sizes)
    pat = [(0,) * nslots] * n_exp
    for e, ab in zip(live_idx, pat_live):
        pat[e] = ab
    slot_chunks = tuple(_split512(p) for p in padded)

    # assign expert segments to slots: slot_lists[i] = [(e, lo, hi), ...]
    slot_lists = [[] for _ in range(nslots)]
    for e in range(n_exp):
        lo, hi = int(starts[e]), int(starts[e + 1])
        for i in range(nslots):
            for _ in range(pat[e][i]):
                take = min(sizes[i], hi - lo)
                slot_lists[i].append((e, lo, lo + take))
                lo += take
        assert lo == hi, "slot solve failed to cover expert"
    for sl in slot_lists:
        while len(sl) < NCORES:
            sl.append((0, 0, 0))

    xT = x.T.astype(ml_dtypes.bfloat16)          # [H, T], contiguous
    w1pk, w2pk = _pack_weights(w1, w2)

    nc = _get_nc(slot_chunks)
    LAST_RESULTS.clear()

    in_maps = []
    for core in range(NCORES):
        m = {}
        for i in range(nslots):
            e, lo, hi = slot_lists[i][core]
            L = padded[i]
            n_s = hi - lo
            xg = np.zeros((H, L), dtype=ml_dtypes.bfloat16)
            cg = np.zeros((L,), dtype=np.float32)
            if n_s:
                xg[:, :n_s] = xT[:, token_sorted[lo:hi]]
                cg[:n_s] = gate_sorted[lo:hi]
            # pack k-tile pairs: [KH//2, 128, 2L], row = [k-even|k-odd]
            m[f"x{i}"] = np.ascontiguousarray(
                xg.reshape(KH // 2, 2, P, L).transpose(0, 2, 1, 3)
            ).reshape(KH // 2, P, 2 * L)
            m[f"c{i}"] = np.ascontiguousarray(np.broadcast_to(cg, (P, L)))
            m[f"w1_{i}"] = w1pk[e]
            m[f"w2_{i}"] = w2pk[e]
        in_maps.append(m)

    try:
        res = run_bass_kernel_spmd(nc, in_maps, list(range(NCORES)))
    except Exception:
        # transient device wedge has been observed to clear on retry
        time.sleep(2)
        res = run_bass_kernel_spmd(nc, in_maps, list(range(NCORES)))
    LAST_RESULTS.append(res)

    out = np.zeros((n_tok, hidden), dtype=np.float32)
    for core in range(NCORES):
        for i in range(nslots):
            e, lo, hi = slot_lists[i][core]
            n_s = hi - lo
            if n_s:
                y = res.results[core][f"y{i}"]       # [H, L] f32, scaled
                # tokens unique within one expert's list -> fancy add ok
                out[token_sorted[lo:hi]] += y[:, :n_s].T
    return out
